# revision 9
# baseline (speedup 1.0000x reference)
"""Chamfer distance kernel for Trainium2 (8 NeuronCores, SPMD).

Reference computation:
    p1 = pc1.reshape(-1, 3)  [N1=16384, 3]
    p2 = pc2.reshape(-1, 3)  [N2=16384, 3]
    d[i, j] = ||p1_i - p2_j||
    out = mean_j(min_i d[i,j]) + mean_i(min_j d[i,j])

Strategy:
  - Shard pc2 rows across 8 cores (2048 points each). Each core computes
    its full distance tile against all of pc1, in both orientations:
      A: [pc1-block=128 part, pc2-shard=2048 free] -> free-min = partial
         col-min (dist2 path), all-min'd across cores on the host.
      B: [pc2-block=128 part, pc1=16384 free]      -> free-min = complete
         row-min (dist1 path) for this core's shard.
  - sqrt is monotone, so mins are taken on squared distances; sqrt and the
    two means run on the host over 8*(16384+2048) partial mins (tiny).
  - d2[i,j] = sq1[i] + sq2[j] - 2*dot(p1_i, p2_j) is produced directly by
    one K=13 augmented matmul per tile: 9 rows give the hi/lo-compensated
    bf16 dot product (error ~2^-16 instead of bf16's 2^-8), 4 rows add
    sq1/sq2 (each split hi+lo). PSUM then holds full fp32 d2 and the DVE
    min-reduce over the free dim finishes each tile.
  - Matmul operands must sit at a 32-partition base, so the 8 pc1
    column-groups of 13 contraction rows are packed at bases {0,32,64,96}
    x 2 column-halves of a [128, 4096] SBUF tensor (also full-width DMA);
    the small pc2-side operands are replicated at all 4 bases.
"""

import os
import sys

import numpy as np

for _p in ("/opt/trn_rl_repo",):
    if os.path.isdir(_p) and _p not in sys.path:
        sys.path.append(_p)

import ml_dtypes

import concourse.bass as bass
import concourse.mybir as mybir
import concourse.tile as tile
from concourse.bass_utils import run_bass_kernel_spmd

BF16 = ml_dtypes.bfloat16

N_CORES = 8
N1 = 16384            # total pc1 points
N_SHARD = 2048        # pc2 points per core
N_GROUPS = 8          # pc1 column-groups
GROUP_COLS = N1 // N_GROUPS  # 2048
K = 24                # augmented contraction depth
MM_N = 512            # matmul moving free dim (one PSUM bank of fp32)
IN_COLS = 6 * GROUP_COLS + 2 * N_SHARD  # packed input columns (16384)

TRACE = False         # test harness can flip this for profiled runs
LAST_RESULTS = None   # stashed BassKernelResults for the test harness

_NC_CACHE = None


def _build_nc():
    """Build the per-core Bass module (same NEFF on all 8 cores)."""
    nc = bass.Bass(trn_type="TRN2")

    # Single packed input (one DMA -> one semaphore for every matmul wait):
    # cols [0:6144) p1w, [6144:12288) p1m, [12288:14336) p2w, [14336:16384) p2m.
    inp = nc.dram_tensor("inp", [128, IN_COLS], mybir.dt.bfloat16,
                         kind="ExternalInput")
    # Single packed output (one DMA -> one tail-drain wait):
    # mout[:, 0:128]  = m2: m2[p, bi] = min over this core's pc2 shard of
    #                   d2(pc1[bi*128+p], .)
    # mout[:, 128:144] = m1: m1[p, bj] = min over all pc1 of
    #                   d2(pc2_shard[bj*128+p], .)
    mout = nc.dram_tensor("mout", [128, N1 // 128 + N_SHARD // 128],
                          mybir.dt.float32, kind="ExternalOutput")

    with tile.TileContext(nc) as tc:
        with (
            tc.tile_pool(name="ins", bufs=1) as ins_pool,
            tc.tile_pool(name="psum", bufs=2, space="PSUM") as psum_pool,
            tc.tile_pool(name="outs", bufs=1) as out_pool,
            tc.tile_pool(name="mins", bufs=2) as mins_pool,
        ):
            inp_sb = ins_pool.tile([128, IN_COLS], mybir.dt.bfloat16,
                                   tag="inp")
            nc.sync.dma_start(inp_sb[:], inp[:])
            p1w_sb = inp_sb[:, 0:3 * GROUP_COLS]
            p1m_sb = inp_sb[:, 3 * GROUP_COLS:6 * GROUP_COLS]
            p2w_sb = inp_sb[:, 6 * GROUP_COLS:6 * GROUP_COLS + N_SHARD]
            p2m_sb = inp_sb[:, 6 * GROUP_COLS + N_SHARD:IN_COLS]

            mout_sb = out_pool.tile([128, N1 // 128 + N_SHARD // 128],
                                    mybir.dt.float32, tag="mout")
            m2_sb = mout_sb[:, 0:N1 // 128]
            m1_sb = mout_sb[:, N1 // 128:N1 // 128 + N_SHARD // 128]

            def grp(sb, g, c0, c1):
                """K-row slice of a group-packed pc1-side tensor."""
                q, h = g % 3, g // 3
                return sb[32 * q:32 * q + K, h * GROUP_COLS + c0:h * GROUP_COLS + c1]

            def rep(sb, g, c0, c1):
                """K-row slice of a base-replicated pc2-side tensor."""
                q = g % 3
                return sb[32 * q:32 * q + K, c0:c1]

            # Orientation A: 128 pc1-blocks; free dim = pc2 shard (2048).
            for bi in range(N1 // 128):
                g, b_in = divmod(bi, GROUP_COLS // 128)
                lhsT = grp(p1w_sb, g, b_in * 128, (b_in + 1) * 128)
                pt = psum_pool.tile([128, N_SHARD], mybir.dt.float32, tag="ps")
                for c in range(N_SHARD // MM_N):
                    nc.tensor.matmul(
                        pt[:, c * MM_N:(c + 1) * MM_N],
                        lhsT,
                        rep(p2m_sb, g, c * MM_N, (c + 1) * MM_N),
                        start=True, stop=True,
                    )
                nc.vector.tensor_reduce(
                    out=m2_sb[:, bi:bi + 1], in_=pt[:],
                    axis=mybir.AxisListType.X, op=mybir.AluOpType.min,
                )

            # Orientation B: 16 pc2-blocks; free dim = all pc1 (8 groups x 2048).
            for bj in range(N_SHARD // 128):
                gmins = mins_pool.tile([128, N_GROUPS], mybir.dt.float32,
                                       tag="gmins")
                for g in range(N_GROUPS):
                    lhsT = rep(p2w_sb, g, bj * 128, (bj + 1) * 128)
                    pt = psum_pool.tile([128, GROUP_COLS], mybir.dt.float32,
                                        tag="ps")
                    for c in range(GROUP_COLS // MM_N):
                        nc.tensor.matmul(
                            pt[:, c * MM_N:(c + 1) * MM_N],
                            lhsT,
                            grp(p1m_sb, g, c * MM_N, (c + 1) * MM_N),
                            start=True, stop=True,
                        )
                    nc.vector.tensor_reduce(
                        out=gmins[:, g:g + 1], in_=pt[:],
                        axis=mybir.AxisListType.X, op=mybir.AluOpType.min,
                    )
                nc.vector.tensor_reduce(
                    out=m1_sb[:, bj:bj + 1], in_=gmins[:],
                    axis=mybir.AxisListType.X, op=mybir.AluOpType.min,
                )

            nc.sync.dma_start(mout[:], mout_sb[:])

    _strip_redundant_pe_waits(nc)
    return nc


def _strip_redundant_pe_waits(nc):
    """Walrus's MM/TR instruction structs carry at most one sem-wait, but
    Tile's sem assignment puts two on the first instruction touching a
    recycled tile slot: a cross-engine wait on the slot's previous
    consumer plus a same-engine wait on its previous producer. Engines
    execute in order (and the cross-engine consumer transitively waited
    on those same-engine ticks), so same-engine waits are redundant:
    drop them whenever a cross-engine wait remains."""
    for blk in nc.m.functions[0].blocks:
        for ins in blk.instructions:
            if type(ins).__name__ not in ("InstMatmult", "InstTensorReduce",
                                          "InstTensorScalarPtr",
                                          "InstTensorTensor", "InstTensorCopy",
                                          "InstActivation"):
                continue
            si = ins.sync_info
            if si is None or len(si.on_wait) <= 1 or not si.on_update:
                continue
            self_eng = si.on_update[0].ant_name.split("_")[0]
            keep = [w for w in si.on_wait
                    if w.ant_name.split("_")[0] != self_eng]
            if not keep or len(keep) == len(si.on_wait):
                continue
            si.on_wait = keep
            ins.sync_info = si
    # The kernel-tail Drain waits on every DMA queue + PE + DVE, but its
    # struct carries very few waits. Everything is transitively covered
    # by the single output DMA (out-DMA waits on the last reduces, which
    # wait on the last matmuls, which waited on the input DMA), so keep
    # only the output DMA queue's wait.
    out_sems = set()
    for blk in nc.m.functions[0].blocks:
        for ins in blk.instructions:
            if type(ins).__name__ == "InstDMACopy" and ins.outs and                     getattr(ins.outs[0], "memref", "") == "mout":
                si = ins.sync_info
                for u in (si.on_update if si else []):
                    out_sems.add(u.ant_name)
    assert out_sems, "output DMA semaphore not found"
    for blk in nc.m.functions[0].blocks:
        for ins in blk.instructions:
            if type(ins).__name__ != "InstDrain":
                continue
            si = ins.sync_info
            if si is None or len(si.on_wait) <= 1:
                continue
            keep = [w for w in si.on_wait if w.ant_name in out_sems]
            if keep and len(keep) < len(si.on_wait):
                si.on_wait = keep
                ins.sync_info = si


def _split3(x):
    """fp32 -> three bf16 terms with x ~= h + m + l (residual ~2^-24 |x|)."""
    h = x.astype(BF16)
    r = x - h.astype(np.float32)
    m = r.astype(BF16)
    l = (r - m.astype(np.float32)).astype(BF16)
    return h, m, l


def _prep_side(p):
    """p: [N, 3] fp32 -> (weight_rows [24, N], moving_rows [24, N]).

    Row r of the weight side pairs with row r of the other cloud's moving
    side; the contraction sums, per coordinate, the six hi/mid/lo product
    terms of magnitude >= ~2^-17 (double-compensated bf16 dot, error
    ~2.5e-7), plus three hi/mid/lo rows for each side's |p|^2."""
    x, y, z = p[:, 0], p[:, 1], p[:, 2]
    sq = (x * x + y * y + z * z).astype(np.float32)
    w_rows, m_rows = [], []
    for c in (x, y, z):
        h, m, l = _split3(c)
        # (W, M) pairs: (h,h) (m,h) (h,m) (l,h) (m,m) (h,l)
        w_rows += [-2 * h, -2 * m, -2 * h, -2 * l, -2 * m, -2 * h]
        m_rows += [h, h, m, h, m, l]
    sh, sm, sl = _split3(sq)
    ones = np.ones_like(sh)
    w_rows += [ones, ones, ones, sh, sm, sl]
    m_rows += [sh, sm, sl, ones, ones, ones]
    return (np.stack(w_rows).astype(BF16), np.stack(m_rows).astype(BF16))


def _group_pack(rows13):
    """[13, N1] -> [128, 6144]: group g at partition base 32*(g%3),
    column region g//3 (AP base partition must be in {0,32,64})."""
    out = np.zeros((128, 3 * GROUP_COLS), dtype=BF16)
    for g in range(N_GROUPS):
        q, h = g % 3, g // 3
        out[32 * q:32 * q + K, h * GROUP_COLS:(h + 1) * GROUP_COLS] = \
            rows13[:, g * GROUP_COLS:(g + 1) * GROUP_COLS]
    return out


def _rep_pack(rows13):
    """[13, N_SHARD] -> [128, N_SHARD]: replicated at bases 0/32/64."""
    out = np.zeros((128, N_SHARD), dtype=BF16)
    for q in range(3):
        out[32 * q:32 * q + K, :] = rows13
    return out


def kernel(pc1, pc2):
    global _NC_CACHE, LAST_RESULTS
    p1 = np.asarray(pc1, dtype=np.float32).reshape(-1, 3)
    p2 = np.asarray(pc2, dtype=np.float32).reshape(-1, 3)
    assert p1.shape == (N1, 3) and p2.shape == (N_CORES * N_SHARD, 3)

    w1, m1rows = _prep_side(p1)
    p1w_np = _group_pack(w1)
    p1m_np = _group_pack(m1rows)

    in_maps = []
    for c in range(N_CORES):
        shard = p2[c * N_SHARD:(c + 1) * N_SHARD]
        w2, m2rows = _prep_side(shard)
        packed = np.concatenate(
            [p1w_np, p1m_np, _rep_pack(w2), _rep_pack(m2rows)], axis=1)
        in_maps.append({"inp": np.ascontiguousarray(packed)})

    if _NC_CACHE is None:
        _NC_CACHE = _build_nc()

    res = run_bass_kernel_spmd(
        _NC_CACHE, in_maps, core_ids=list(range(N_CORES)), trace=TRACE,
    )
    LAST_RESULTS = res

    # m1 per core: complete row-mins of d2 for its 2048 pc2 points.
    # m2 per core: partial col-mins of d2 over its shard -> min across cores.
    nb2 = N1 // 128
    d2_1 = np.concatenate(
        [r["mout"][:, nb2:].T.reshape(-1) for r in res.results])  # [16384] pc2-major
    d2_2 = np.min(
        np.stack([r["mout"][:, :nb2].T.reshape(-1) for r in res.results]),
        axis=0)                                                   # [16384]

    dist1 = np.sqrt(np.maximum(d2_1, 0.0))
    dist2 = np.sqrt(np.maximum(d2_2, 0.0))
    return np.float32(dist1.mean() + dist2.mean())


# revision 10
# speedup vs baseline: 1.0049x; 1.0049x over previous
"""Chamfer distance kernel for Trainium2 (8 NeuronCores, SPMD).

Reference computation:
    p1 = pc1.reshape(-1, 3)  [N1=16384, 3]
    p2 = pc2.reshape(-1, 3)  [N2=16384, 3]
    d[i, j] = ||p1_i - p2_j||
    out = mean_j(min_i d[i,j]) + mean_i(min_j d[i,j])

Strategy:
  - Shard pc2 rows across 8 cores (2048 points each). Each core computes
    its full distance tile against all of pc1, in both orientations:
      A: [pc1-block=128 part, pc2-shard=2048 free] -> free-min = partial
         col-min (dist2 path), all-min'd across cores on the host.
      B: [pc2-block=128 part, pc1=16384 free]      -> free-min = complete
         row-min (dist1 path) for this core's shard.
  - sqrt is monotone, so mins are taken on squared distances; sqrt and the
    two means run on the host over 8*(16384+2048) partial mins (tiny).
  - d2[i,j] = sq1[i] + sq2[j] - 2*dot(p1_i, p2_j) is produced directly by
    one K=13 augmented matmul per tile: 9 rows give the hi/lo-compensated
    bf16 dot product (error ~2^-16 instead of bf16's 2^-8), 4 rows add
    sq1/sq2 (each split hi+lo). PSUM then holds full fp32 d2 and the DVE
    min-reduce over the free dim finishes each tile.
  - Matmul operands must sit at a 32-partition base, so the 8 pc1
    column-groups of 13 contraction rows are packed at bases {0,32,64,96}
    x 2 column-halves of a [128, 4096] SBUF tensor (also full-width DMA);
    the small pc2-side operands are replicated at all 4 bases.
"""

import os
import sys

import numpy as np

for _p in ("/opt/trn_rl_repo",):
    if os.path.isdir(_p) and _p not in sys.path:
        sys.path.append(_p)

import ml_dtypes

import concourse.bass as bass
import concourse.mybir as mybir
import concourse.tile as tile
from concourse.bass_utils import run_bass_kernel_spmd

BF16 = ml_dtypes.bfloat16

N_CORES = 8
N1 = 16384            # total pc1 points
N_SHARD = 2048        # pc2 points per core
N_GROUPS = 8          # pc1 column-groups
GROUP_COLS = N1 // N_GROUPS  # 2048
K = 24                # augmented contraction depth
MM_N = 512            # matmul moving free dim (one PSUM bank of fp32)
IN_COLS = 6 * GROUP_COLS + 2 * N_SHARD  # packed input columns (16384)

TRACE = False         # test harness can flip this for profiled runs
LAST_RESULTS = None   # stashed BassKernelResults for the test harness

_NC_CACHE = None


def _build_nc():
    """Build the per-core Bass module (same NEFF on all 8 cores)."""
    nc = bass.Bass(trn_type="TRN2")

    # Single packed input (one DMA -> one semaphore for every matmul wait):
    # cols [0:6144) p1w, [6144:12288) p1m, [12288:14336) p2w, [14336:16384) p2m.
    inp = nc.dram_tensor("inp", [128, IN_COLS], mybir.dt.bfloat16,
                         kind="ExternalInput")
    # Single packed output (one DMA -> one tail-drain wait):
    # mout[:, 0:128]  = m2: m2[p, bi] = min over this core's pc2 shard of
    #                   d2(pc1[bi*128+p], .)
    # mout[:, 128:144] = m1: m1[p, bj] = min over all pc1 of
    #                   d2(pc2_shard[bj*128+p], .)
    mout = nc.dram_tensor("mout", [128, N1 // 128 + N_SHARD // 128],
                          mybir.dt.float32, kind="ExternalOutput")

    with tile.TileContext(nc) as tc:
        with (
            tc.tile_pool(name="ins", bufs=1) as ins_pool,
            tc.tile_pool(name="psum", bufs=2, space="PSUM") as psum_pool,
            tc.tile_pool(name="outs", bufs=1) as out_pool,
            tc.tile_pool(name="mins", bufs=2) as mins_pool,
        ):
            inp_sb = ins_pool.tile([128, IN_COLS], mybir.dt.bfloat16,
                                   tag="inp")
            nc.sync.dma_start(inp_sb[:], inp[:])
            p1w_sb = inp_sb[:, 0:3 * GROUP_COLS]
            p1m_sb = inp_sb[:, 3 * GROUP_COLS:6 * GROUP_COLS]
            p2w_sb = inp_sb[:, 6 * GROUP_COLS:6 * GROUP_COLS + N_SHARD]
            p2m_sb = inp_sb[:, 6 * GROUP_COLS + N_SHARD:IN_COLS]

            mout_sb = out_pool.tile([128, N1 // 128 + N_SHARD // 128],
                                    mybir.dt.float32, tag="mout")
            m2_sb = mout_sb[:, 0:N1 // 128]
            m1_sb = mout_sb[:, N1 // 128:N1 // 128 + N_SHARD // 128]

            def grp(sb, g, c0, c1):
                """K-row slice of a group-packed pc1-side tensor."""
                q, h = g % 3, g // 3
                return sb[32 * q:32 * q + K, h * GROUP_COLS + c0:h * GROUP_COLS + c1]

            def rep(sb, g, c0, c1):
                """K-row slice of a base-replicated pc2-side tensor."""
                q = g % 3
                return sb[32 * q:32 * q + K, c0:c1]

            # Orientation A: 128 pc1-blocks; free dim = pc2 shard (2048).
            for bi in range(N1 // 128):
                g, b_in = divmod(bi, GROUP_COLS // 128)
                lhsT = grp(p1w_sb, g, b_in * 128, (b_in + 1) * 128)
                pt = psum_pool.tile([128, N_SHARD], mybir.dt.float32, tag="ps")
                for c in range(N_SHARD // MM_N):
                    nc.tensor.matmul(
                        pt[:, c * MM_N:(c + 1) * MM_N],
                        lhsT,
                        rep(p2m_sb, g, c * MM_N, (c + 1) * MM_N),
                        start=True, stop=True,
                    )
                nc.vector.tensor_reduce(
                    out=m2_sb[:, bi:bi + 1], in_=pt[:],
                    axis=mybir.AxisListType.X, op=mybir.AluOpType.min,
                )

            # Orientation B: 16 pc2-blocks; free dim = all pc1 (8 groups x 2048).
            for bj in range(N_SHARD // 128):
                gmins = mins_pool.tile([128, N_GROUPS], mybir.dt.float32,
                                       tag="gmins")
                for g in range(N_GROUPS):
                    lhsT = rep(p2w_sb, g, bj * 128, (bj + 1) * 128)
                    pt = psum_pool.tile([128, GROUP_COLS], mybir.dt.float32,
                                        tag="ps")
                    for c in range(GROUP_COLS // MM_N):
                        nc.tensor.matmul(
                            pt[:, c * MM_N:(c + 1) * MM_N],
                            lhsT,
                            grp(p1m_sb, g, c * MM_N, (c + 1) * MM_N),
                            start=True, stop=True,
                        )
                    nc.vector.tensor_reduce(
                        out=gmins[:, g:g + 1], in_=pt[:],
                        axis=mybir.AxisListType.X, op=mybir.AluOpType.min,
                    )
                nc.vector.tensor_reduce(
                    out=m1_sb[:, bj:bj + 1], in_=gmins[:],
                    axis=mybir.AxisListType.X, op=mybir.AluOpType.min,
                )

            nc.sync.dma_start(mout[:], mout_sb[:])

    _strip_redundant_pe_waits(nc)
    _elide_repeated_ldweights(nc)
    return nc


def _elide_repeated_ldweights(nc):
    """Mark matmuls that reuse the previous matmul's stationary operand
    with ldweights=False so the PE skips the redundant weight load
    (walrus's own ldw-opt pass is disabled in this toolchain). PE
    executes a block's instructions in list order, so comparing against
    the previous PE matmul in that order is exact."""
    n = 0
    for blk in nc.m.functions[0].blocks:
        prev_w = None
        for ins in blk.instructions:
            if type(ins).__name__ != "InstMatmult":
                continue
            w = ins.ins[1]
            key = (w.memref, w.offset, tuple(tuple(x) for x in w.ap))
            if prev_w == key:
                ins.ldweights = False
                n += 1
            prev_w = key
    assert n > 0, "no repeated-weight matmuls found"


def _strip_redundant_pe_waits(nc):
    """Walrus's MM/TR instruction structs carry at most one sem-wait, but
    Tile's sem assignment puts two on the first instruction touching a
    recycled tile slot: a cross-engine wait on the slot's previous
    consumer plus a same-engine wait on its previous producer. Engines
    execute in order (and the cross-engine consumer transitively waited
    on those same-engine ticks), so same-engine waits are redundant:
    drop them whenever a cross-engine wait remains."""
    for blk in nc.m.functions[0].blocks:
        for ins in blk.instructions:
            if type(ins).__name__ not in ("InstMatmult", "InstTensorReduce",
                                          "InstTensorScalarPtr",
                                          "InstTensorTensor", "InstTensorCopy",
                                          "InstActivation"):
                continue
            si = ins.sync_info
            if si is None or len(si.on_wait) <= 1 or not si.on_update:
                continue
            self_eng = si.on_update[0].ant_name.split("_")[0]
            keep = [w for w in si.on_wait
                    if w.ant_name.split("_")[0] != self_eng]
            if not keep or len(keep) == len(si.on_wait):
                continue
            si.on_wait = keep
            ins.sync_info = si
    # The kernel-tail Drain waits on every DMA queue + PE + DVE, but its
    # struct carries very few waits. Everything is transitively covered
    # by the single output DMA (out-DMA waits on the last reduces, which
    # wait on the last matmuls, which waited on the input DMA), so keep
    # only the output DMA queue's wait.
    out_sems = set()
    for blk in nc.m.functions[0].blocks:
        for ins in blk.instructions:
            if type(ins).__name__ == "InstDMACopy" and ins.outs and                     getattr(ins.outs[0], "memref", "") == "mout":
                si = ins.sync_info
                for u in (si.on_update if si else []):
                    out_sems.add(u.ant_name)
    assert out_sems, "output DMA semaphore not found"
    for blk in nc.m.functions[0].blocks:
        for ins in blk.instructions:
            if type(ins).__name__ != "InstDrain":
                continue
            si = ins.sync_info
            if si is None or len(si.on_wait) <= 1:
                continue
            keep = [w for w in si.on_wait if w.ant_name in out_sems]
            if keep and len(keep) < len(si.on_wait):
                si.on_wait = keep
                ins.sync_info = si


def _split3(x):
    """fp32 -> three bf16 terms with x ~= h + m + l (residual ~2^-24 |x|)."""
    h = x.astype(BF16)
    r = x - h.astype(np.float32)
    m = r.astype(BF16)
    l = (r - m.astype(np.float32)).astype(BF16)
    return h, m, l


def _prep_side(p):
    """p: [N, 3] fp32 -> (weight_rows [24, N], moving_rows [24, N]).

    Row r of the weight side pairs with row r of the other cloud's moving
    side; the contraction sums, per coordinate, the six hi/mid/lo product
    terms of magnitude >= ~2^-17 (double-compensated bf16 dot, error
    ~2.5e-7), plus three hi/mid/lo rows for each side's |p|^2."""
    x, y, z = p[:, 0], p[:, 1], p[:, 2]
    sq = (x * x + y * y + z * z).astype(np.float32)
    w_rows, m_rows = [], []
    for c in (x, y, z):
        h, m, l = _split3(c)
        # (W, M) pairs: (h,h) (m,h) (h,m) (l,h) (m,m) (h,l)
        w_rows += [-2 * h, -2 * m, -2 * h, -2 * l, -2 * m, -2 * h]
        m_rows += [h, h, m, h, m, l]
    sh, sm, sl = _split3(sq)
    ones = np.ones_like(sh)
    w_rows += [ones, ones, ones, sh, sm, sl]
    m_rows += [sh, sm, sl, ones, ones, ones]
    return (np.stack(w_rows).astype(BF16), np.stack(m_rows).astype(BF16))


def _group_pack(rows13):
    """[13, N1] -> [128, 6144]: group g at partition base 32*(g%3),
    column region g//3 (AP base partition must be in {0,32,64})."""
    out = np.zeros((128, 3 * GROUP_COLS), dtype=BF16)
    for g in range(N_GROUPS):
        q, h = g % 3, g // 3
        out[32 * q:32 * q + K, h * GROUP_COLS:(h + 1) * GROUP_COLS] = \
            rows13[:, g * GROUP_COLS:(g + 1) * GROUP_COLS]
    return out


def _rep_pack(rows13):
    """[13, N_SHARD] -> [128, N_SHARD]: replicated at bases 0/32/64."""
    out = np.zeros((128, N_SHARD), dtype=BF16)
    for q in range(3):
        out[32 * q:32 * q + K, :] = rows13
    return out


def kernel(pc1, pc2):
    global _NC_CACHE, LAST_RESULTS
    p1 = np.asarray(pc1, dtype=np.float32).reshape(-1, 3)
    p2 = np.asarray(pc2, dtype=np.float32).reshape(-1, 3)
    assert p1.shape == (N1, 3) and p2.shape == (N_CORES * N_SHARD, 3)

    w1, m1rows = _prep_side(p1)
    p1w_np = _group_pack(w1)
    p1m_np = _group_pack(m1rows)

    in_maps = []
    for c in range(N_CORES):
        shard = p2[c * N_SHARD:(c + 1) * N_SHARD]
        w2, m2rows = _prep_side(shard)
        packed = np.concatenate(
            [p1w_np, p1m_np, _rep_pack(w2), _rep_pack(m2rows)], axis=1)
        in_maps.append({"inp": np.ascontiguousarray(packed)})

    if _NC_CACHE is None:
        _NC_CACHE = _build_nc()

    res = run_bass_kernel_spmd(
        _NC_CACHE, in_maps, core_ids=list(range(N_CORES)), trace=TRACE,
    )
    LAST_RESULTS = res

    # m1 per core: complete row-mins of d2 for its 2048 pc2 points.
    # m2 per core: partial col-mins of d2 over its shard -> min across cores.
    nb2 = N1 // 128
    d2_1 = np.concatenate(
        [r["mout"][:, nb2:].T.reshape(-1) for r in res.results])  # [16384] pc2-major
    d2_2 = np.min(
        np.stack([r["mout"][:, :nb2].T.reshape(-1) for r in res.results]),
        axis=0)                                                   # [16384]

    dist1 = np.sqrt(np.maximum(d2_1, 0.0))
    dist2 = np.sqrt(np.maximum(d2_2, 0.0))
    return np.float32(dist1.mean() + dist2.mean())
